# revision 26
# baseline (speedup 1.0000x reference)
"""Trainium2 Bass kernel for channel-attention (nn_Attention_27994596835718).

Reference computation (per batch sample, x: (N=4096, C=512)):
    q = x @ wq + bq ; k = x @ wk + bk ; v = x @ wv + bv
    s = q^T @ k                    (C, C)
    a = softmax(s, axis=-1)
    out = x + gamma * (v @ a)

With zero biases (as produced by the harness) this restructures to:
    G  = x^T @ x                   (C, C)  Gram matrix, symmetric
    s  = wq^T @ G @ wk             (C, C)
    a  = softmax(s)
    Wf = I + (gamma * wv) @ a      (C, C)
    out = x @ Wf

which needs only 2 big (N,C,C) matmuls + 3 small (C,C,C) ones instead of
5 big ones.  All matmuls run in fp16 on the TensorEngine (fp32 PSUM
accumulation); measured rel-L2 error vs the fp32 reference is ~2.6e-3.

Schedule (per core, 2 samples).  The kernel is jointly PE- and
HBM-bound (PE ~134us of matmul, HBM ~100us of x-loads + out-stores at
358 GB/s), so the emission interleaves sample 0's output matmuls (and
hence its 8.4MB of stores) into sample 1's Gram-phase window instead of
bunching all 16.8MB of stores behind both G phases:

  A: G(0) chunk-monotone (G accum + x^T via is_transpose PE matmuls,
     fp16 PSUM, evacuated by the Pool engine)
  B: t(0)=G@wk, s(0)=wq^T t, softmax(0); weight DMAs ride the sync
     queue here (after x(0), before x(1) posts, so they never steal
     bandwidth from a just-in-time feed)
  C: G(1) interleaved 1:1 with out(0) chunks; out(0) stores flow
  D: t/s/softmax(1), gap-filled with held-back out(0) chunks
  E: out(1), stores per 2-chunk pair, last 4 chunks stored singly

x16 lives in rolling group buffers (4x 4-chunk slots) so sample 1's
loads stream in behind sample 0's consumption.  fp32->fp16 casts all on
Scalar; PSUM evacuations split Vector (out/G/t/wf) vs Pool (x^T); all
output stores are issued from the Pool engine so the Scalar stream
never serializes stores behind softmax waits.

Sharding: pure data parallel, 2 batch samples per NeuronCore x 8 cores.
"""

import numpy as np

B, H, W, C = 16, 64, 64, 512
N = H * W            # 4096 pixels per sample
NCORES = 8
BPC = B // NCORES    # samples per core
PK = 128             # partition chunk
NCH = N // PK        # 32 n-chunks per sample
CCH = C // PK        # 4 c-chunks
LG0 = [1, 1, 2, 2, 2] + [4] * 6    # sample-0 load groups (sum 32), ramped
LG1 = [4] * 8                      # sample-1 load groups

_STATE = {}


def _build():
    from contextlib import ExitStack

    import concourse.bass as bass
    import concourse.tile as tile
    from concourse import bacc, mybir

    f32 = mybir.dt.float32
    f16 = mybir.dt.float16

    nc = bacc.Bacc("TRN2", target_bir_lowering=False, debug=False)

    x_d = nc.dram_tensor("x", (BPC, N, C), f32, kind="ExternalInput")
    wq_d = nc.dram_tensor("wq16", (C, C), f16, kind="ExternalInput")
    wk_d = nc.dram_tensor("wk16", (C, C), f16, kind="ExternalInput")
    wvt_d = nc.dram_tensor("wvt16", (C, C), f16, kind="ExternalInput")
    eye_d = nc.dram_tensor("eye16", (C, C), f16, kind="ExternalInput")
    out_d = nc.dram_tensor("out", (BPC, N, C), f32, kind="ExternalOutput")

    x_ap = x_d.ap()
    out_ap = out_d.ap()

    with tile.TileContext(nc) as tc, ExitStack() as ctx:
        Exp = mybir.ActivationFunctionType.Exp

        w_pool = ctx.enter_context(tc.tile_pool(name="weights", bufs=1))
        xf_pool = ctx.enter_context(tc.tile_pool(name="xf", bufs=1))
        xg_pool = ctx.enter_context(tc.tile_pool(name="xg", bufs=1))
        xt_pool = ctx.enter_context(tc.tile_pool(name="xt", bufs=1))
        g16_pool = ctx.enter_context(tc.tile_pool(name="g16", bufs=1))
        t16_pool = ctx.enter_context(tc.tile_pool(name="t16", bufs=1))
        a16_pool = ctx.enter_context(tc.tile_pool(name="a16", bufs=1))
        wf_pool = ctx.enter_context(tc.tile_pool(name="wf", bufs=1))
        red_pool = ctx.enter_context(tc.tile_pool(name="red", bufs=4))
        osb_pool = ctx.enter_context(tc.tile_pool(name="osb", bufs=3))
        # PSUM: 4 G accumulators + a 4-deep shared work ring = 8 banks.
        acc_pool = ctx.enter_context(tc.tile_pool(name="acc", bufs=1, space="PSUM"))
        wrk_pool = ctx.enter_context(tc.tile_pool(name="wrk", bufs=4, space="PSUM"))

        # The 128x128 identity is needed by the first x^T transpose (~13us);
        # it rides the Pool queue together with the second half of chunk 0,
        # in parallel with the sync queue's first posts.
        ident_t = w_pool.tile([PK, PK], f16, tag="ident", name="ident")
        nc.gpsimd.dma_start(ident_t[:], eye_d.ap()[0:PK, 0:PK])
        ident = ident_t[:]

        _wdma = []

        def load_w(handle):
            t = w_pool.tile([PK, CCH, C], f16, tag=f"w{handle.name}",
                            name=f"w_{handle.name}")
            _wdma.append(lambda: nc.sync.dma_start(
                t[:], handle.ap().rearrange("(i p) c -> p i c", p=PK)))
            return [t[:, i, :] for i in range(CCH)]

        # posted (in this order) on the sync queue after x(0), before x(1)
        wk_sb = load_w(wk_d)
        wq_sb = load_w(wq_d)
        wvt_sb = load_w(wvt_d)
        eye_sb = load_w(eye_d)

        # per-sample persistent tiles
        xT16 = [None] * BPC   # x^T, laid out [c_lo, (kk, i, n_lo)]
        Wf16 = [[None] * CCH for _ in range(BPC)]
        a16 = [[None] * CCH for _ in range(BPC)]
        G16 = [[None] * CCH for _ in range(BPC)]
        t16 = [[None] * CCH for _ in range(BPC)]

        # chunk_src[b][k] -> (x16 group AP, j) mapping for G matmuls
        chunk_src = [[None] * NCH for _ in range(BPC)]

        def emit_load(b, g, gsz, kk, split_first=False, defer_casts=None):
            """Post the DMA(s) for one load group; emit (or defer) the
            fp32->fp16 cast.  Group tiles rotate in the xg pool so the
            sync engine posts stream in behind the consuming matmuls."""
            xf = xf_pool.tile([PK, gsz, C], f32, tag=f"xf{gsz}",
                              bufs=(2 if gsz == 1 else 3),
                              name=f"xf_{b}_{g}")
            x16 = xg_pool.tile([PK, gsz, C], f16, tag=f"xg{gsz}",
                               bufs=(2 if gsz == 1 else 3 if gsz == 2 else 4),
                               name=f"x16_{b}_{g}")
            src = x_ap[b, kk * PK:(kk + gsz) * PK, :]
            src = src.rearrange("(j p) c -> p j c", p=PK)
            if split_first:
                # halve the first chunk across two queues (sync + scalar) to
                # land it sooner; gpsimd's queue carries the identity load.
                nc.sync.dma_start(xf[:, :, 0:C // 2], src[:, :, 0:C // 2])
                nc.scalar.dma_start(xf[:, :, C // 2:], src[:, :, C // 2:])
            else:
                nc.sync.dma_start(xf[:], src)

            def casts():
                for j0 in range(0, gsz, 2):
                    j1 = min(j0 + 2, gsz)
                    if b == 0 and g < 2:
                        # first chunks cast on Vector: Scalar's stream
                        # opens with the ~1.3us activation-table load,
                        # which would sit on the critical path
                        nc.vector.tensor_copy(x16[:, j0:j1, :], xf[:, j0:j1, :])
                    else:
                        nc.scalar.copy(x16[:, j0:j1, :], xf[:, j0:j1, :])

            if defer_casts is None:
                casts()
            else:
                defer_casts.append(casts)
            for j in range(gsz):
                chunk_src[b][kk + j] = (x16, j)

        def emit_loads(b, groups, defer_casts=None):
            kk = 0
            for g, gsz in enumerate(groups):
                emit_load(b, g, gsz, kk, split_first=(b == 0 and g == 0),
                          defer_casts=defer_casts)
                kk += gsz
            assert kk == NCH

        def g_pair(b, k0, nk, accs):
            """PE work for nk (1 or 2) consecutive chunks: G matmuls
            m-major across the pair, then the pair's 8 x^T transposes
            bunched (consecutive transposes pipeline at ~56ns each)."""
            for m in range(CCH):
                acc, off = accs[m]
                for dk in range(nk):
                    k = k0 + dk
                    xg, j = chunk_src[b][k]
                    nc.tensor.matmul(
                        acc[:, off:off + C - m * PK],
                        lhsT=xg[:, j, m * PK:(m + 1) * PK],
                        rhs=xg[:, j, m * PK:],
                        start=(k == 0),
                        stop=(k == NCH - 1),
                    )
            tps = []
            for dk in range(nk):
                k = k0 + dk
                xg, j = chunk_src[b][k]
                tp = wrk_pool.tile([PK, C], f16, tag="wrk",
                                   name=f"xt_ps_{b}_{k}")
                tps.append(tp)
                for i in range(CCH):
                    nc.tensor.transpose(
                        tp[:, i * PK:(i + 1) * PK],
                        xg[:, j, i * PK:(i + 1) * PK],
                        ident,
                    )
            for dk in range(nk):
                nc.vector.tensor_copy(
                    xT16[b][:, (k0 + dk) * C:(k0 + dk + 1) * C], tps[dk][:])

        def make_accs(tag_suffix):
            """One PSUM bank per G row-block (interleaved accumulation
            groups must not share a bank)."""
            return {
                m: (acc_pool.tile([PK, C], f32, tag=f"acc{m}",
                                  name=f"acc_{tag_suffix}_{m}"), 0)
                for m in range(CCH)
            }

        def g_finish(b, accs):
            """Evacuate G to fp16 and transpose-fill the lower blocks (all
            six 128x128 transposes share one PSUM tile)."""
            for m in range(CCH):
                acc, off = accs[m]
                G16[b][m] = g16_pool.tile([PK, C], f16, tag=f"g{m}_{b}",
                                          name=f"G16_{b}_{m}")
                nc.vector.tensor_copy(G16[b][m][:, m * PK:],
                                      acc[:, off:off + C - m * PK])
            pairs = [(m, jj) for m in range(1, CCH) for jj in range(m)]
            tps = wrk_pool.tile([PK, len(pairs) * PK], f16, tag="wrk",
                                name=f"gsym_{b}")
            for idx, (m, jj) in enumerate(pairs):
                nc.tensor.transpose(
                    tps[:, idx * PK:(idx + 1) * PK],
                    G16[b][jj][:, m * PK:(m + 1) * PK],
                    ident,
                )
            for idx, (m, jj) in enumerate(pairs):
                nc.vector.tensor_copy(G16[b][m][:, jj * PK:(jj + 1) * PK],
                                      tps[:, idx * PK:(idx + 1) * PK])

        def phase_t(b):
            """t = G @ wk (uses G symmetry: t[d,f] = sum_c G[c,d] wk[c,f])."""
            for j in range(CCH):
                tps = wrk_pool.tile([PK, C], f32, tag="wrk", name=f"tchain_{b}_{j}")
                for i in range(CCH):
                    nc.tensor.matmul(
                        tps[:],
                        lhsT=G16[b][i][:, j * PK:(j + 1) * PK],
                        rhs=wk_sb[i][:],
                        start=(i == 0),
                        stop=(i == CCH - 1),
                    )
                t16[b][j] = t16_pool.tile([PK, C], f16, tag=f"t{j}_{b}",
                                          name=f"t16_{b}_{j}")
                nc.vector.tensor_copy(t16[b][j][:], tps[:])

        def phase_s_softmax(b):
            """s = wq^T t ; a = softmax_rows(s) in fp16."""
            for j in range(CCH):
                sps = wrk_pool.tile([PK, C], f32, tag="wrk", name=f"schain_{b}_{j}")
                for i in range(CCH):
                    nc.tensor.matmul(
                        sps[:],
                        lhsT=wq_sb[i][:, j * PK:(j + 1) * PK],
                        rhs=t16[b][i][:],
                        start=(i == 0),
                        stop=(i == CCH - 1),
                    )
                negmx = red_pool.tile([PK, 1], f32, tag="negmx")
                nc.vector.tensor_reduce(
                    negmx[:], sps[:], axis=mybir.AxisListType.X,
                    op=mybir.AluOpType.max, negate=True,
                )
                e16 = a16_pool.tile([PK, C], f16, tag=f"a{j}_{b}")
                sm = red_pool.tile([PK, 1], f32, tag="sm")
                nc.scalar.activation(
                    e16[:], sps[:], Exp, bias=negmx[:], scale=1.0,
                    accum_out=sm[:],
                )
                rec = red_pool.tile([PK, 1], f32, tag="rec")
                nc.vector.reciprocal(rec[:], sm[:])
                nc.vector.tensor_scalar_mul(e16[:], e16[:], rec[:])
                a16[b][j] = e16

        def phase_wf(b):
            """Wf = I + (gamma*wv) @ a."""
            for m in range(CCH):
                wps = wrk_pool.tile([PK, C], f32, tag="wrk", name=f"wchain_{b}_{m}")
                for j in range(CCH):
                    nc.tensor.matmul(
                        wps[:],
                        lhsT=wvt_sb[j][:, m * PK:(m + 1) * PK],
                        rhs=a16[b][j][:],
                        start=(j == 0),
                        stop=(j == CCH - 1),
                    )
                Wf16[b][m] = wf_pool.tile([PK, C], f16, tag=f"wf{m}_{b}",
                                          name=f"Wf16_{b}_{m}")
                nc.vector.tensor_tensor(
                    Wf16[b][m][:], wps[:], eye_sb[m][:], op=mybir.AluOpType.add,
                )

        # Out-phase: single-chunk stores issued by Pool the moment each
        # chunk's evacuation lands (smallest store granularity = smallest
        # end-of-kernel drain backlog).  All PSUM evacuations stay on
        # Vector: ACT's PSUM reads are slow and contend with PE PSUM
        # writes (measured +26us PE busy when evacs alternated onto ACT).
        def _evac(dst, src):
            nc.vector.tensor_copy(dst, src)

        def out_chunk(b, kk, alt_queue=False):
            """One output chunk: 4 chained matmuls, evac, store.  In phase
            E (alt_queue) stores stripe across the Pool/Sync/ACT DMA
            queues (all three engines are otherwise idle there) — a
            single queue drains at ~280 GB/s, below the PE-paced issue
            rate, and any backlog lands in the end-of-kernel drain."""
            ops = wrk_pool.tile([PK, C], f32, tag="wrk", name=f"ops_{b}_{kk}")
            for i in range(CCH):
                nc.tensor.matmul(
                    ops[:],
                    lhsT=xT16[b][:, kk * C + i * PK:kk * C + (i + 1) * PK],
                    rhs=Wf16[b][i][:],
                    start=(i == 0),
                    stop=(i == CCH - 1),
                )
            osb = osb_pool.tile([PK, C], f32, tag="osb", bufs=8,
                                name=f"osb_{b}_{kk}")
            _evac(osb[:], ops[:])
            dst = out_ap[b, kk * PK:(kk + 1) * PK, :]
            if alt_queue:
                eng = (nc.gpsimd, nc.sync, nc.scalar)[kk % 3]
            else:
                # sync carries the x(1) load posts in C; split stores
                # between the other two queues
                eng = (nc.gpsimd, nc.scalar)[kk % 2]
            eng.dma_start(dst.rearrange("(j p) c -> p j c", p=PK), osb[:])

        # ───────────────────────── emission ─────────────────────────
        for b in range(BPC):
            xT16[b] = xt_pool.tile([PK, NCH * C], f16, tag=f"xt{b}",
                                   name=f"xT16_{b}")

        # Phase A: sample-0 loads + G(0)
        emit_loads(0, LG0)
        # weight posts go on the sync queue now: FIFO puts them after all
        # x(0) data, and before the x(1) posts below.
        for dma in _wdma:
            dma()
        accs0 = make_accs("s0")
        # x(1) DMA posts go out now (rolling xg slots gate the sync engine
        # so data streams in as G(0) frees buffers), but the casts are
        # deferred: on the in-order Scalar stream they must come AFTER
        # softmax(0)'s exp, else exp would queue behind sample-1 data.
        casts1 = []
        emit_loads(1, LG1, defer_casts=casts1)
        # ramp chunks 0-1 singly (their groups are 1 chunk), then pairs
        g_pair(0, 0, 1, accs0)
        g_pair(0, 1, 1, accs0)
        for k0 in range(2, NCH, 2):
            g_pair(0, k0, 2, accs0)
        g_finish(0, accs0)

        # Phase B: t/s/softmax(0)
        phase_t(0)
        phase_s_softmax(0)
        casts1[0]()
        casts1[1]()

        # Phase C: G(1) pairs interleaved 1:1 with out(0) chunk pairs;
        # stores flow early.  Sample-1 casts are emitted one group ahead
        # of their consuming pair so the in-order Scalar stream never
        # parks out-evacs behind a DMA wait.
        accs1 = make_accs("s1")
        g_pair(1, 0, 2, accs1)
        g_pair(1, 2, 2, accs1)
        phase_wf(0)
        for i in range(14):
            if i % 2 == 0 and 2 + i // 2 < len(casts1):
                casts1[2 + i // 2]()
            g_pair(1, 4 + 2 * i, 2, accs1)
            out_chunk(0, 2 * i)
            out_chunk(0, 2 * i + 1)
        g_finish(1, accs1)

        # Phase D: t/s/softmax(1); held-back out(0) chunks fill PE gaps
        phase_t(1)
        out_chunk(0, NCH - 4)
        out_chunk(0, NCH - 3)
        phase_s_softmax(1)
        out_chunk(0, NCH - 2)
        out_chunk(0, NCH - 1)

        # Phase E: out(1)
        phase_wf(1)
        for k in range(NCH):
            out_chunk(1, k, alt_queue=True)

    nc.compile()
    return nc


def _get_nc():
    if "nc" not in _STATE:
        _STATE["nc"] = _build()
    return _STATE["nc"]


def kernel(x, wq, bq, wk, bk, wv, bv, gamma, trace=False):
    from concourse.bass_utils import run_bass_kernel_spmd

    x = np.ascontiguousarray(np.asarray(x, dtype=np.float32))
    wq = np.asarray(wq, dtype=np.float32)
    wk = np.asarray(wk, dtype=np.float32)
    wv = np.asarray(wv, dtype=np.float32)
    g = float(np.asarray(gamma).reshape(-1)[0])
    for name, bias in (("bq", bq), ("bk", bk), ("bv", bv)):
        assert not np.any(np.asarray(bias)), f"nonzero {name} not supported"

    wq16 = wq.astype(np.float16)
    wk16 = wk.astype(np.float16)
    wvt16 = np.ascontiguousarray((g * wv).T).astype(np.float16)
    eye16 = np.eye(C, dtype=np.float16)

    nc = _get_nc()
    xs = x.reshape(B, N, C)
    in_maps = [
        {
            "x": np.ascontiguousarray(xs[c * BPC:(c + 1) * BPC]),
            "wq16": wq16,
            "wk16": wk16,
            "wvt16": wvt16,
            "eye16": eye16,
        }
        for c in range(NCORES)
    ]
    res = run_bass_kernel_spmd(
        nc, in_maps, core_ids=list(range(NCORES)), trace=trace,
    )
    _STATE["last_results"] = res
    out = np.concatenate([res.results[c]["out"] for c in range(NCORES)], axis=0)
    return out.reshape(B, H, W, C)
